# revision 10
# baseline (speedup 1.0000x reference)
"""Local-window (banded) multi-head attention on 8 Trainium2 NeuronCores.

Problem: x[L=2048, B=8, D=512], Wqkv[1536, 512], Wout[512, 512], bout[512].
  qkv = x @ Wqkv.T ; per-head banded attention (|i-j| <= 64, window 129);
  out = attn_out @ Wout.T + bout.

Sharding: batch B=8 across the 8 cores (data parallel). Each core runs the
full pipeline for one batch element.

Fast path (fp8 DoubleRow):
  - Q/K projections run as 2-term fp8 DoubleRow matmuls (x split hi+lo fp8
    against W*2^6 in fp8; the 2^-6 compensation is folded into the mandatory
    PSUM->SBUF drain, which becomes a tensor_scalar multiply at copy cost).
  - q,k are stored fp8. The banded-score matmul is ONE fp8 DoubleRow
    instruction per 128x128 block: pair0 contracts K^T (64 dh rows, zero
    padded to 128) against Q, pair1 contracts a 128-identity against a
    0/-240 mask tile, adding the band mask into the score PSUM for free.
    exp(0.125*(s-240)) underflows fp16 to exact 0 out-of-band.
  - V projection is 3-term fp8 (x hi+lo x Wv hi+lo, dropping lo*lo) with
    Wv scaled 2^6; the compensation is folded into the softmax denominator
    by using 64.0 as the "ones" vector of the denominator matmul.
  - V-proj writes the 64-shifted key layout directly by offsetting the
    stationary x window, so each V tile drains in a single full-width copy.
  - Everything else (PV, out-projection, exp output) is fp16, which is
    free accuracy over bf16 at identical PE cost.
  - Normalization is a Pool-engine multiply with a stride-0 broadcast AP of
    the per-(query,head) reciprocals; bias folds into the out-proj drain.
"""

import sys

import numpy as np

if "/opt/trn_rl_repo" not in sys.path:
    sys.path.insert(0, "/opt/trn_rl_repo")

L, B, D, H, DH = 2048, 8, 512, 8, 64
WIN, PAD = 129, 64
NCH = L // 128  # 16 query chunks of 128
KTW = 64 + L + 64  # padded K^T columns (col = key + 64)
WSC = 6  # log2 weight prescale for fp8
QK_TERMS = 2  # fp8 split terms for Q/K projection (2 or 3)

_NC_CACHE = {}


def _build_nc():
    from concourse import bacc, mybir, tile
    import concourse.bass as bass

    f32 = mybir.dt.float32
    fp16 = mybir.dt.float16
    fp8 = mybir.dt.float8e4
    DR = mybir.MatmulPerfMode.DoubleRow
    Exp = mybir.ActivationFunctionType.Exp

    nc = bacc.Bacc(None, target_bir_lowering=False)

    xh_d = nc.dram_tensor("xh", [128, 4 * L], fp8, kind="ExternalInput")
    xl_d = nc.dram_tensor("xl", [128, 4 * L], fp8, kind="ExternalInput")
    # Q,K weights (cols: Q 0..511, K 512..1023), ktile-major, scaled 2^WSC
    wqk_d = nc.dram_tensor("wqk", [128, 4 * 2 * D], fp8, kind="ExternalInput")
    wqkl_d = nc.dram_tensor("wqkl", [128, 4 * 2 * D], fp8, kind="ExternalInput")
    wvh_d = nc.dram_tensor("wvh", [128, 4 * D], fp8, kind="ExternalInput")
    wvl_d = nc.dram_tensor("wvl", [128, 4 * D], fp8, kind="ExternalInput")
    woutT_d = nc.dram_tensor("woutT", [D, D], fp16, kind="ExternalInput")
    bout_d = nc.dram_tensor("bout", [D], f32, kind="ExternalInput")
    # 4 mask variants [128,128] (mid_r0, mid_r1, first_r0, last_r1) + identity
    maskI_d = nc.dram_tensor("maskI", [128, 5 * 128], fp8, kind="ExternalInput")
    kz_d = nc.dram_tensor("kz", [KTW], fp8, kind="ExternalInput")
    y_d = nc.dram_tensor("y", [L, D], f32, kind="ExternalOutput")

    with tile.TileContext(nc) as tc, nc.allow_low_precision(
        reason="fp8 DoubleRow operands; accumulation stays fp32 in PSUM"
    ):
        with (
            tc.tile_pool(name="pers", bufs=1) as pers,
            tc.tile_pool(name="ps", bufs=1, space="PSUM") as ps,
        ):
            # ---- persistent SBUF tensors ----
            xh = pers.tile([128, 4 * L], fp8, name="xh", tag="xh")
            xl = pers.tile([128, 4 * L], fp8, name="xl", tag="xl")
            wqk = pers.tile([128, 8 * D], fp8, name="wqk", tag="wqk")
            wqkl = (
                pers.tile([128, 8 * D], fp8, name="wqkl", tag="wqkl")
                if QK_TERMS >= 3
                else None
            )
            wvh = pers.tile([128, 4 * D], fp8, name="wvh", tag="wvh")
            wvl = pers.tile([128, 4 * D], fp8, name="wvl", tag="wvl")
            wos = [
                pers.tile([128, D], fp16, name=f"wo{k}", tag=f"wo{k}") for k in range(4)
            ]
            boutb = pers.tile([128, D], f32, name="boutb", tag="boutb")
            onesc = pers.tile([128, 1], fp16, name="onesc", tag="onesc")
            # Q^T per head pair + 4 mask tiles appended at cols L..L+511
            QT = [
                pers.tile([128, L + 512], fp8, name=f"QT{t}", tag=f"QT{t}")
                for t in range(4)
            ]
            # K^T per head (64 live dh rows + 64 zero rows) + identity at KTW..
            KT = [
                pers.tile([128, KTW + 128], fp8, name=f"KT{h}", tag=f"KT{h}")
                for h in range(H)
            ]
            Vs = [
                pers.tile([128, D], fp16, name=f"Vs{j}", tag=f"Vs{j}")
                for j in range(NCH + 1)
            ]

            def mm(out, lhsT, rhs, start, stop, pm=None):
                nc.tensor.matmul(out, lhsT, rhs, start=start, stop=stop, perf_mode=pm)

            def pair_ap(base_ap, pair_stride):
                return bass.AP(
                    tensor=base_ap.tensor,
                    offset=base_ap.offset,
                    ap=[list(base_ap.ap[0]), [pair_stride, 2], list(base_ap.ap[-1])],
                )

            # ---- Pool-side constant init + PE warm-up (Pool queue FIRST:
            # everything else on Pool would delay the warm-up memset) ----
            warm = pers.tile([128, 512], fp16, name="warm", tag="warm")
            nc.gpsimd.memset(warm[:], 0.0)
            wp = ps.tile([128, 512], f32, name="warmp", tag="big", bufs=2)
            for wi in range(5):
                mm(wp[:], warm[:, 0:128], warm[:], start=True, stop=True)
            nc.gpsimd.memset(onesc[:], float(2**WSC))
            ident = pers.tile([128, 128], fp16, name="ident", tag="ident")
            nc.gpsimd.memset(ident[:], 1.0)
            nc.gpsimd.affine_select(
                out=ident[:], in_=ident[:], compare_op=mybir.AluOpType.is_equal,
                fill=0.0, base=0, pattern=[[-1, 128]], channel_multiplier=1,
            )
            ones_row = pers.tile([1, 128], fp16, name="ones_row", tag="ones_row")
            nc.gpsimd.memset(ones_row[:], 1.0)
            nc.gpsimd.memset(Vs[0][0:64, :], 0.0)
            nc.gpsimd.memset(Vs[NCH][64:128, :], 0.0)

            # ---- input DMAs: interleave SP/Act HWDGE rings (Pool's software
            # DGE costs ~580ns of Pool seq per DMA), first-needed first ----
            dma_i = 0

            def dma(dst, src):
                nonlocal dma_i
                eng = nc.sync if dma_i % 2 == 0 else nc.scalar
                dma_i += 1
                eng.dma_start(out=dst, in_=src)

            # starters: window-0 x hi + Q weights, then x lo + K weights
            for k in range(4):
                dma(xh[:, k * L : k * L + 512], xh_d[:, k * L : k * L + 512])
                dma(wqk[:, k * 1024 : k * 1024 + 512], wqk_d[:, k * 1024 : k * 1024 + 512])
            for k in range(4):
                dma(xl[:, k * L : k * L + 512], xl_d[:, k * L : k * L + 512])
                dma(
                    wqk[:, k * 1024 + 512 : k * 1024 + 1024],
                    wqk_d[:, k * 1024 + 512 : k * 1024 + 1024],
                )
            if QK_TERMS >= 3:
                nc.sync.dma_start(out=wqkl[:], in_=wqkl_d[:, :])
            # V weights next: prologue V tiles gate chunk-0/1 PV
            dma(wvh[:], wvh_d[:, :])
            dma(wvl[:], wvl_d[:, :])
            # masks into QT tiles, identity + zero pads into KT tiles
            for t in range(4):
                dma(QT[t][:, L : L + 512], maskI_d[:, 0:512])
            for h in range(H):
                dma(KT[h][:, KTW : KTW + 128], maskI_d[:, 512:640])
            kz_ap = kz_d[:]
            for h in range(H):
                dead = slice(64, 128) if h % 2 == 0 else slice(0, 64)
                live = slice(0, 64) if h % 2 == 0 else slice(64, 128)
                dma(
                    KT[h][dead, 0:KTW],
                    bass.AP(
                        tensor=kz_ap.tensor, offset=kz_ap.offset, ap=[[0, 64], [1, KTW]]
                    ),
                )
                dma(
                    KT[h][live, 0:64],
                    bass.AP(
                        tensor=kz_ap.tensor, offset=kz_ap.offset, ap=[[0, 64], [1, 64]]
                    ),
                )
                dma(
                    KT[h][live, 64 + L : KTW],
                    bass.AP(
                        tensor=kz_ap.tensor, offset=kz_ap.offset, ap=[[0, 64], [1, 64]]
                    ),
                )
            # x remainder: window-1 slice first (feeds chunk 0-2 fills),
            # then the rest; both rings
            for k in range(4):
                dma(
                    xh[:, k * L + 512 : k * L + 1024],
                    xh_d[:, k * L + 512 : k * L + 1024],
                )
                dma(
                    xl[:, k * L + 512 : k * L + 1024],
                    xl_d[:, k * L + 512 : k * L + 1024],
                )
            for k in range(4):
                dma(
                    xh[:, k * L + 1024 : (k + 1) * L],
                    xh_d[:, k * L + 1024 : (k + 1) * L],
                )
            for k in range(4):
                dma(
                    xl[:, k * L + 1024 : (k + 1) * L],
                    xl_d[:, k * L + 1024 : (k + 1) * L],
                )
            for k in range(4):
                nc.sync.dma_start(out=wos[k][:], in_=woutT_d[k * 128 : (k + 1) * 128, :])
            bout_ap = bout_d[:]
            nc.scalar.dma_start(
                out=boutb[:],
                in_=bass.AP(
                    tensor=bout_ap.tensor, offset=bout_ap.offset, ap=[[0, 128], [1, D]]
                ),
            )
            bout_row = pers.tile([1, D], fp16, name="bout_row", tag="bout_row")
            nc.gpsimd.tensor_copy(out=bout_row[:], in_=boutb[0:1, :])

            # ---- projection emitters (PE filler during attention) ----
            drain_i = 0

            def qk_drain(dst, src):
                # scaled drain: PSUM f32 * 2^-WSC -> SBUF fp8
                nonlocal drain_i
                drain_i += 1
                nc.vector.tensor_scalar_mul(out=dst, in0=src, scalar1=float(2.0**-WSC))

            def emit_q(t, w, half=None):
                qp = ps.tile([128, 512], f32, name=f"qp{t}_{w}_{half}", tag="big", bufs=2)
                halves = (half,) if half is not None else (0, 1)
                for hf in halves:
                    terms = [(xh, wqk), (xl, wqk)]
                    if QK_TERMS >= 3:
                        terms.append((xh, wqkl))
                    n = 0
                    for xt, wt in terms:
                        for s in range(2):
                            lw = wt[:, 2 * s * 1024 + t * 128 : 2 * s * 1024 + t * 128 + 128]
                            rx = xt[
                                :,
                                2 * s * L + 512 * w + 256 * hf : 2 * s * L
                                + 512 * w
                                + 256 * hf
                                + 256,
                            ]
                            mm(
                                qp[:, 256 * hf : 256 * hf + 256],
                                pair_ap(lw, 1024),
                                pair_ap(rx, L),
                                start=(n == 0),
                                stop=(n == 2 * len(terms) - 1),
                                pm=DR,
                            )
                            n += 1
                if half is None:
                    qk_drain(QT[t][:, 512 * w : 512 * w + 512], qp[:])
                else:
                    qk_drain(
                        QT[t][:, 512 * w + 256 * half : 512 * w + 256 * half + 256],
                        qp[:, 256 * half : 256 * half + 256],
                    )

            def emit_k(t, w, half=None):
                kp = ps.tile([128, 512], f32, name=f"kp{t}_{w}_{half}", tag="big", bufs=2)
                halves = (half,) if half is not None else (0, 1)
                for hf in halves:
                    terms = [(xh, wqk), (xl, wqk)]
                    if QK_TERMS >= 3:
                        terms.append((xh, wqkl))
                    n = 0
                    for xt, wt in terms:
                        for s in range(2):
                            c0 = 2 * s * 1024 + 512 + t * 128
                            lw = wt[:, c0 : c0 + 128]
                            rx = xt[
                                :,
                                2 * s * L + 512 * w + 256 * hf : 2 * s * L
                                + 512 * w
                                + 256 * hf
                                + 256,
                            ]
                            mm(
                                kp[:, 256 * hf : 256 * hf + 256],
                                pair_ap(lw, 1024),
                                pair_ap(rx, L),
                                start=(n == 0),
                                stop=(n == 2 * len(terms) - 1),
                                pm=DR,
                            )
                            n += 1
                if half is not None and half == 0:
                    return kp
                kq = pers.tile([128, D], fp8, name=f"ks{t}_{w}", tag="kst", bufs=2)
                qk_drain(kq[:], kp[:])
                # scatter halves into the two per-head padded K^T tiles
                cs = slice(64 + 512 * w, 64 + 512 * w + 512)
                nc.sync.dma_start(out=KT[2 * t][0:64, cs], in_=kq[0:64, :])
                nc.sync.dma_start(out=KT[2 * t + 1][64:128, cs], in_=kq[64:128, :])
                return None

            def emit_v(lt):
                vp = ps.tile([128, 512], f32, name=f"vp{lt}", tag="big", bufs=2)
                x0 = 128 * lt - 64
                if lt == 0:
                    # fp8 DR matmuls may not target PSUM partition base 64:
                    # compute keys 0..63 at base 0, then partition-shift the
                    # drain into Vs[0][64:128] with an SBUF->SBUF DMA.
                    xw, p0, pw = 0, 0, 64
                elif lt == NCH:
                    xw, p0, pw = L - 64, 0, 64
                else:
                    xw, p0, pw = x0, 0, 128
                terms = [(xh, wvh), (xl, wvh), (xh, wvl)]
                for dh in range(2):
                    n = 0
                    for xt, wt in terms:
                        for s in range(2):
                            lx = xt[:, 2 * s * L + xw : 2 * s * L + xw + pw]
                            rw = wt[
                                :, 2 * s * 512 + 256 * dh : 2 * s * 512 + 256 * dh + 256
                            ]
                            mm(
                                vp[p0 : p0 + pw, 256 * dh : 256 * dh + 256],
                                pair_ap(lx, L),
                                pair_ap(rw, 512),
                                start=(n == 0),
                                stop=(n == 5),
                                pm=DR,
                            )
                            n += 1
                # Pool cannot read PSUM on HW: V drains go to Act
                if lt == 0:
                    vst = pers.tile([64, D], fp16, name="vst0", tag="vst0")
                    nc.scalar.copy(out=vst[:], in_=vp[0:64, :])
                    nc.sync.dma_start(out=Vs[0][64:128, :], in_=vst[:])
                else:
                    nc.scalar.copy(out=Vs[lt][p0 : p0 + pw, :], in_=vp[p0 : p0 + pw, :])

            # fill-group schedule: window w of Q/K (8 groups) spread over
            # chunks 4(w-1)..4(w-1)+2; V tiles trail their first use.
            fill_groups = {ch: [] for ch in range(NCH)}
            for w in range(1, 4):
                base = 4 * (w - 1)
                sched = [3, 3, 2]
                gi = 0
                for off, cnt in enumerate(sched):
                    for _ in range(cnt):
                        t, which = gi % 4, gi // 4
                        fill_groups[base + off].append(
                            (emit_q if which == 0 else emit_k, (t, w))
                        )
                        gi += 1
            for lt in range(2, NCH + 1):
                fill_groups[min(lt - 2, 12)].append((emit_v, (lt,)))
            for c in (13, 14, 15):
                fill_groups[c].append((None, (c - 1, [0, 1])))
                fill_groups[c].append((None, (c - 1, [2, 3])))
                fill_groups[c].append(("store", (c - 1,)))

            # ---- prologue projections: Q/K window 0 + V tiles 0,1 ----
            for t in range(4):
                emit_q(t, 0, half=0)
                emit_q(t, 0, half=1)
            for t in range(4):
                emit_k(t, 0)
            emit_v(0)
            emit_v(1)

            # ---- main loop ----
            otts = [None] * NCH
            yps = {}

            def emit_outproj_mms(ch, ts):
                if ch not in yps:
                    yps[ch] = ps.tile([128, 512], f32, name=f"yp{ch}", tag="big", bufs=2)
                for t in ts:
                    mm(
                        yps[ch][:],
                        otts[ch][:, t * 128 : (t + 1) * 128],
                        wos[t][:],
                        start=(t == 0),
                        stop=(t == 3),
                    )

            def emit_outproj_store(ch, split=False):
                yp = yps[ch]
                ysb = pers.tile([128, D], f32, name=f"ysb{ch}", tag="ysb", bufs=2)
                if split:
                    for hf in range(2):
                        cs = slice(256 * hf, 256 * (hf + 1))
                        nc.vector.tensor_add(
                            out=ysb[:, cs], in0=yp[:, cs], in1=boutb[:, cs]
                        )
                        eng = nc.sync if hf == 0 else nc.scalar
                        eng.dma_start(
                            out=y_d[ch * 128 : (ch + 1) * 128, cs], in_=ysb[:, cs]
                        )
                else:
                    nc.vector.tensor_add(out=ysb[:], in0=yp[:], in1=boutb[:])
                    nc.sync.dma_start(out=y_d[ch * 128 : (ch + 1) * 128, :], in_=ysb[:])

            def emit_outproj(ch):
                emit_outproj_mms(ch, range(4))
                emit_outproj_store(ch)

            for ch in range(NCH):
                fills = fill_groups[ch]
                fi = 0

                def fill(n):
                    nonlocal fi
                    for _ in range(min(n, len(fills) - fi)):
                        fn, args = fills[fi]
                        if fn is None:
                            emit_outproj_mms(*args)
                        elif fn == "store":
                            emit_outproj_store(*args)
                        else:
                            fn(*args)
                        fi += 1

                # fused banded scores + mask: one fp8 DR matmul per block
                pts = []
                for pg in range(2):
                    scp = ps.tile([128, 1024], f32, name=f"sc{ch}_{pg}", tag="sc", bufs=2)
                    for bi in range(8):
                        hd = 4 * pg + bi // 2
                        r = bi % 2
                        c0 = 128 * (ch + r)
                        if ch == 0 and r == 0:
                            mi = 2
                        elif ch == NCH - 1 and r == 1:
                            mi = 3
                        else:
                            mi = r
                        lk = KT[hd][:, c0 : c0 + 128]
                        rq = QT[hd // 2][:, 128 * ch : 128 * ch + 128]
                        mm(
                            scp[:, bi * 128 : (bi + 1) * 128],
                            pair_ap(lk, KTW - c0),
                            pair_ap(rq, L + 128 * mi - 128 * ch),
                            start=True,
                            stop=True,
                            pm=DR,
                        )
                    pt = pers.tile([128, 1024], fp16, name=f"pt{ch}_{pg}", tag="pt", bufs=3)
                    nc.scalar.activation(out=pt[:], in_=scp[:], func=Exp, scale=0.125)
                    pts.append(pt)

                fill(1)
                opc = ps.tile([128, 512], f32, name=f"op{ch}", tag="op", bufs=1)
                dnc = ps.tile([128, 8], f32, name=f"dn{ch}", tag="dn", bufs=1)
                for hd in range(H):
                    pt = pts[hd // 4]
                    bi = 2 * (hd % 4)
                    for r in range(2):
                        blk = pt[:, (bi + r) * 128 : (bi + r + 1) * 128]
                        mm(
                            dnc[:, hd : hd + 1], blk, onesc[:],
                            start=(r == 0), stop=(r == 1),
                        )
                        mm(
                            opc[:, 64 * hd : 64 * (hd + 1)],
                            blk,
                            Vs[ch + r][:, 64 * hd : 64 * (hd + 1)],
                            start=(r == 0),
                            stop=(r == 1),
                        )
                    if hd % 2 == 1 and hd < 7:
                        fill(1)

                # normalization: reciprocal + Pool broadcast multiply
                otq = pers.tile([128, 512], fp16, name=f"otq{ch}", tag="otq", bufs=2)
                ott = pers.tile([128, 512], fp16, name=f"ott{ch}", tag="ott", bufs=3)
                otts[ch] = ott
                if ch >= NCH - 3:
                    # tail chunks: half-granular norm + transpose so consumer
                    # out-projections unblock earlier
                    for hf in range(2):
                        cs = slice(256 * hf, 256 * (hf + 1))
                        rrh = pers.tile([128, 4], f32, name=f"rr{ch}_{hf}", tag="rrh", bufs=2)
                        nc.vector.reciprocal(out=rrh[:], in_=dnc[:, 4 * hf : 4 * (hf + 1)])
                        rbh = pers.tile([128, 256], f32, name=f"rb{ch}_{hf}", tag="rbh", bufs=2)
                        rr_ap = rrh[:]
                        rr_b = bass.AP(
                            tensor=rr_ap.tensor,
                            offset=rr_ap.offset,
                            ap=[list(rr_ap.ap[0]), [1, 4], [0, 64]],
                        )
                        nc.gpsimd.tensor_copy(
                            out=rbh[:].rearrange("p (h e) -> p h e", e=64), in_=rr_b
                        )
                        nc.vector.tensor_mul(out=otq[:, cs], in0=opc[:, cs], in1=rbh[:])
                        if ch < NCH - 1:
                            nc.sync.dma_start_transpose(
                                out=ott[:, cs].rearrange("p (t q) -> p t q", q=128),
                                in_=otq[:, cs],
                            )
                    if ch >= NCH - 1:
                        for t in range(4):
                            tp = ps.tile(
                                [128, 128], fp16, name=f"tp{ch}_{t}", tag="big", bufs=2
                            )
                            nc.tensor.transpose(
                                out=tp[:],
                                in_=otq[:, t * 128 : (t + 1) * 128],
                                identity=ident[:],
                            )
                            nc.scalar.copy(out=ott[:, t * 128 : (t + 1) * 128], in_=tp[:])
                else:
                    rr = pers.tile([128, 8], f32, name=f"rr{ch}", tag="rr", bufs=2)
                    nc.vector.reciprocal(out=rr[:], in_=dnc[:])
                    rb = pers.tile([128, 512], f32, name=f"rb{ch}", tag="rb", bufs=2)
                    rr_ap = rr[:]
                    rr_b = bass.AP(
                        tensor=rr_ap.tensor,
                        offset=rr_ap.offset,
                        ap=[list(rr_ap.ap[0]), [1, 8], [0, 64]],
                    )
                    nc.gpsimd.tensor_copy(
                        out=rb[:].rearrange("p (h e) -> p h e", e=64), in_=rr_b
                    )
                    nc.vector.tensor_mul(out=otq[:], in0=opc[:], in1=rb[:])
                    nc.sync.dma_start_transpose(
                        out=ott[:].rearrange("p (t q) -> p t q", q=128), in_=otq[:]
                    )
                fill(len(fills))
                if 1 <= ch <= 12:
                    emit_outproj(ch - 1)

            # final store: fold bout in as a rank-1 PSUM accumulation
            lc = NCH - 1
            emit_outproj_mms(lc, range(3))
            mm(yps[lc][:], otts[lc][:, 384:512], wos[3][:], start=False, stop=False)
            mm(yps[lc][:], ones_row[:], bout_row[:], start=False, stop=True)
            ysbl = pers.tile([128, D], f32, name="ysb_last", tag="ysb", bufs=2)
            for hf in range(2):
                cs = slice(256 * hf, 256 * (hf + 1))
                nc.scalar.copy(out=ysbl[:, cs], in_=yps[lc][:, cs])
                eng = nc.sync if hf == 0 else nc.scalar
                eng.dma_start(out=y_d[lc * 128 : (lc + 1) * 128, cs], in_=ysbl[:, cs])

    nc.compile()
    return nc


def get_nc():
    if "nc" not in _NC_CACHE:
        _NC_CACHE["nc"] = _build_nc()
    return _NC_CACHE["nc"]


def make_core_inputs(x, Wqkv, Wout, bout):
    """Host-side shard + layout prep: fp8 hi/lo splits, ktile-major views."""
    from concourse import mybir

    fp8 = mybir.dt.np(mybir.dt.float8e4)
    fp16 = np.float16

    x = np.asarray(x, dtype=np.float32)
    Wqkv = np.asarray(Wqkv, dtype=np.float32)
    Wout = np.asarray(Wout, dtype=np.float32)
    bout = np.ascontiguousarray(np.asarray(bout, dtype=np.float32))

    def ktile_split(a, ncols):
        # a [512, ncols] f32 -> hi, lo fp8 [128, 4*ncols] (ktile-major)
        hi = a.astype(fp8)
        lo = (a - hi.astype(np.float32)).astype(fp8)

        def kt(m):
            return np.ascontiguousarray(
                m.reshape(4, 128, ncols).transpose(1, 0, 2).reshape(128, 4 * ncols)
            )

        return kt(hi), kt(lo)

    wT = np.ascontiguousarray(Wqkv.T) * float(2**WSC)  # [512, 1536]
    wqk_h, wqk_l = ktile_split(wT[:, 0 : 2 * D], 2 * D)
    wv_h, wv_l = ktile_split(wT[:, 2 * D : 3 * D], D)
    woutT = np.ascontiguousarray(Wout.T).astype(fp16)

    p = np.arange(128)[:, None]
    c = np.arange(128)[None, :]
    masks = np.zeros((128, 5 * 128), dtype=np.float32)
    masks[:, 0:128] = np.where(p >= c, 0.0, -240.0)
    masks[:, 128:256] = np.where(p <= c, 0.0, -240.0)
    masks[:, 256:384] = np.where((p >= c) & (p >= 64), 0.0, -240.0)
    masks[:, 384:512] = np.where((p <= c) & (p <= 63), 0.0, -240.0)
    masks[:, 512:640] = np.eye(128, dtype=np.float32)
    maskI = masks.astype(fp8)
    kz = np.zeros((KTW,), dtype=fp8)

    in_maps = []
    for b in range(B):
        xT = np.ascontiguousarray(x[:, b, :].T)  # [512, 2048]
        xh, xlo = ktile_split(xT, L)
        in_maps.append(
            {
                "xh": xh,
                "xl": xlo,
                "wqk": wqk_h,
                "wqkl": wqk_l,
                "wvh": wv_h,
                "wvl": wv_l,
                "woutT": woutT,
                "bout": bout,
                "maskI": maskI,
                "kz": kz,
            }
        )
    return in_maps


def kernel(x, Wqkv, Wout, bout):
    from concourse.bass_utils import run_bass_kernel_spmd

    nc = get_nc()
    in_maps = make_core_inputs(x, Wqkv, Wout, bout)
    res = run_bass_kernel_spmd(nc, in_maps, core_ids=list(range(B)))
    out = np.empty((L, B, D), dtype=np.float32)
    for b in range(B):
        out[:, b, :] = res.results[b]["y"]
    return out
